# revision 1
# baseline (speedup 1.0000x reference)
"""Banded multi-head attention kernel for Trainium2 (8 NeuronCores).

Problem: q = query @ Wq.T + bq, k = key @ Wk.T + bk  (per head, dk=64),
scores = q.k / sqrt(dk) masked to |i-j| <= 16, softmax over keys, then
gather the 33-column select window per row -> out [B, NH, T, 33].

Strategy:
  - Shard (batch b, half of T) across the 8 cores; each core computes all
    8 heads for its 1024 query rows.
  - Host pre-transposes activations/weights so every matmul contraction
    dim lands on SBUF partitions; inputs load in a handful of large
    batched DMAs (HWDGE fixed cost ~0.5us per dma_start dominates
    otherwise).
  - Device (fp32 throughout): PE matmuls for the projections; per
    (head, 128-row block) one banded score matmul [K=64] x [128, 160]
    (the key window is a contiguous slice in k^T layout); -1e30 band
    mask added on DVE; exp(x/8) on ScalarE into a persistent SBUF band
    region; one wide DVE reduce per row block computes all 8 row-sum
    columns; reciprocal + per-head normalize on DVE; one output DMA per
    row block. (float32r / PSUM-bank pairing / GpSimd offload were
    measured or faulted worse - see flags.)
  - Host: final diagonal gather band -> [T, 33] (pure strided indexing,
    handles the sequence-edge select-window clipping exactly).
"""

import sys

sys.path.insert(0, "/opt/trn_rl_repo")

import numpy as np

B, T, HID = 4, 2048, 512
NH, DK, W = 8, 64, 16
WIN = 2 * W + 1  # 33
TEMP = 8.0
NCORES = 8
THALF = T // 2  # rows per core
NBLK = THALF // 128  # 8 row blocks per core
BAND = 160  # key-window width per 128-row block: 128 + 2*16
KW = THALF + 2 * W  # 1056 k^T columns needed per core
NEG = -1.0e30

F32R_PROJ = False  # float32r projections: compiles but faults at execution
GPSIMD_MULS = False  # measured slower: gpsimd muls sit on the output critical path
PAIR_PSUM = False  # two start=True MMs into one PSUM tile: suspect

_CACHE = {}


def _build_nc():
    import concourse.bass as bass  # noqa: F401
    import concourse.tile as tile
    from concourse import bacc, mybir

    f32 = mybir.dt.float32
    f32r = mybir.dt.float32r
    AF = mybir.ActivationFunctionType

    nc = bacc.Bacc("TRN2", target_bir_lowering=False, debug=False)

    fin = f32r if F32R_PROJ else f32
    qT = nc.dram_tensor("qT", [HID, THALF], fin, kind="ExternalInput").ap()
    kT = nc.dram_tensor("kT", [HID, KW], fin, kind="ExternalInput").ap()
    wqT = nc.dram_tensor("wqT", [HID, HID], fin, kind="ExternalInput").ap()
    wkT = nc.dram_tensor("wkT", [HID, HID], fin, kind="ExternalInput").ap()
    # biases: [:, 0:4] = bq chunks, [:, 4:8] = bk chunks
    bia = nc.dram_tensor("bia", [128, 8], f32, kind="ExternalInput").ap()
    # per block: the band mask duplicated for a head pair (320 cols)
    msk = nc.dram_tensor("msk", [128, NBLK, 2 * BAND], f32, kind="ExternalInput").ap()
    # output band: [p, r, h, n]
    outp = nc.dram_tensor(
        "outp", [128, NBLK, NH, BAND], f32, kind="ExternalOutput"
    ).ap()

    with tile.TileContext(nc) as tc:
        from contextlib import ExitStack

        with ExitStack() as ctx:
            const = ctx.enter_context(tc.tile_pool(name="const", bufs=1))
            psum_p = ctx.enter_context(
                tc.tile_pool(name="psum_p", bufs=2, space="PSUM")
            )
            psum_s = ctx.enter_context(
                tc.tile_pool(name="psum_s", bufs=6, space="PSUM")
            )
            work = ctx.enter_context(tc.tile_pool(name="work", bufs=4))

            qin = const.tile([128, 4, THALF], fin, tag="qin", name="qin")
            kin = const.tile([128, 4, KW], fin, tag="kin", name="kin")
            wq_sb = const.tile([128, 4, HID], fin, tag="wqs", name="wqs")
            wk_sb = const.tile([128, 4, HID], fin, tag="wks", name="wks")
            mk_sb = const.tile([128, NBLK, 2 * BAND], f32, tag="mks", name="mks")
            bia_sb = const.tile([128, 8], f32, tag="bia", name="bias")
            qp = [const.tile([128, THALF], f32, tag=f"qp{i}", name=f"qp{i}")
                  for i in range(4)]
            kp = [const.tile([128, KW], f32, tag=f"kp{i}", name=f"kp{i}")
                  for i in range(4)]
            # persistent output band region [p, r, h, n]
            ob = const.tile([128, NBLK, NH, BAND], f32, tag="ob", name="ob")

            # input loads; activations/weights split per 128-partition chunk
            # so the first projection matmuls can start early
            qT_r = qT.rearrange("(c p) t -> p c t", p=128)
            kT_r = kT.rearrange("(c p) t -> p c t", p=128)
            wqT_r = wqT.rearrange("(c p) o -> p c o", p=128)
            wkT_r = wkT.rearrange("(c p) o -> p c o", p=128)
            nc.sync.dma_start(out=bia_sb[:, :], in_=bia[:, :])
            for ic in range(4):
                nc.sync.dma_start(out=wq_sb[:, ic, :], in_=wqT_r[:, ic, :])
                nc.sync.dma_start(out=qin[:, ic, :], in_=qT_r[:, ic, :])
            for ic in range(4):
                nc.sync.dma_start(out=wk_sb[:, ic, :], in_=wkT_r[:, ic, :])
                nc.sync.dma_start(out=kin[:, ic, :], in_=kT_r[:, ic, :])
            nc.sync.dma_start(out=mk_sb[:, :, :], in_=msk[:, :, :])

            # q projection: q^T[o, t] = sum_i Wq^T[i, o] * query^T[i, t] + bq[o]
            # psum->sbuf (+bias) on DVE to keep ScalarE free for Exp
            for oc in range(4):
                osl = slice(128 * oc, 128 * (oc + 1))
                for tb in range(THALF // 512):
                    tsl = slice(512 * tb, 512 * (tb + 1))
                    ps = psum_p.tile([128, 512], f32, tag="psp", name="psp")
                    for ic in range(4):
                        nc.tensor.matmul(
                            ps[:, :],
                            wq_sb[:, ic, osl],
                            qin[:, ic, tsl],
                            start=(ic == 0),
                            stop=(ic == 3),
                        )
                    nc.vector.tensor_scalar_add(
                        qp[oc][:, tsl], ps[:, :], bia_sb[:, oc : oc + 1]
                    )
            # k projection over 1056 columns: chunks 512/512/32, on ScalarE
            for oc in range(4):
                osl = slice(128 * oc, 128 * (oc + 1))
                for c0, cn in [(0, 512), (512, 512), (1024, KW - 1024)]:
                    ps = psum_p.tile([128, 512], f32, tag="psp", name="psp")
                    for ic in range(4):
                        nc.tensor.matmul(
                            ps[:, :cn],
                            wk_sb[:, ic, osl],
                            kin[:, ic, c0 : c0 + cn],
                            start=(ic == 0),
                            stop=(ic == 3),
                        )
                    nc.scalar.activation(
                        kp[oc][:, c0 : c0 + cn],
                        ps[:, :cn],
                        AF.Identity,
                        bias=bia_sb[:, 4 + oc : 5 + oc],
                        scale=1.0,
                    )

            # banded scores + softmax per (row block, head)
            for r in range(NBLK):
                rs = work.tile([128, NH], f32, tag="rs", name="rs")
                rc = work.tile([128, NH], f32, tag="rc", name="rc")
                for oc in range(4):  # head pair (2*oc, 2*oc+1)
                    for half in range(2):
                        h = 2 * oc + half
                        dsl = slice(64 * half, 64 * (half + 1))
                        ps = psum_s.tile(
                            [128, BAND], f32, tag="pss", name="pss"
                        )
                        nc.tensor.matmul(
                            ps[:, :],
                            qp[oc][dsl, 128 * r : 128 * (r + 1)],
                            kp[oc][dsl, 128 * r : 128 * r + BAND],
                            start=True,
                            stop=True,
                        )
                        sm = work.tile(
                            [128, BAND], f32, tag="sm", name="sm"
                        )
                        nc.vector.tensor_add(
                            sm[:, :], ps[:, :], mk_sb[:, r, :BAND]
                        )
                        nc.scalar.activation(
                            ob[:, r, h, :],
                            sm[:, :],
                            AF.Exp,
                            scale=1.0 / TEMP,
                        )
                # all-head row sums in one wide reduce, then reciprocal
                import concourse.mybir as mybir_  # AxisListType

                nc.vector.tensor_reduce(
                    rs[:, :],
                    ob[:, r, :, :],
                    axis=mybir_.AxisListType.X,
                    op=mybir_.AluOpType.add,
                )
                nc.vector.reciprocal(rc[:, :], rs[:, :])
                for h in range(NH):
                    nc.vector.tensor_scalar_mul(
                        ob[:, r, h, :], ob[:, r, h, :], rc[:, h : h + 1]
                    )
                nc.sync.dma_start(out=outp[:, r, :, :], in_=ob[:, r, :, :])

    nc.compile()
    return nc


def _get_nc():
    if "nc" not in _CACHE:
        _CACHE["nc"] = _build_nc()
    return _CACHE["nc"]


def host_prep(query, key, Wq, bq, Wk, bk):
    """Build the 8 per-core input maps."""
    query = np.ascontiguousarray(np.asarray(query, dtype=np.float32))
    key = np.ascontiguousarray(np.asarray(key, dtype=np.float32))
    Wq = np.asarray(Wq, dtype=np.float32)
    Wk = np.asarray(Wk, dtype=np.float32)
    bq = np.asarray(bq, dtype=np.float32)
    bk = np.asarray(bk, dtype=np.float32)

    wqT = np.ascontiguousarray(Wq.T)
    wkT = np.ascontiguousarray(Wk.T)
    bia = np.ascontiguousarray(
        np.concatenate([bq.reshape(4, 128).T, bk.reshape(4, 128).T], axis=1)
    )

    p = np.arange(128)[:, None]
    n = np.arange(BAND)[None, :]
    in_maps = []
    for c in range(NCORES):
        b, th = c // 2, c % 2
        t0 = th * THALF
        qTs = np.ascontiguousarray(query[b].T[:, t0 : t0 + THALF])
        kTs = np.zeros((HID, KW), np.float32)
        j0 = t0 - W
        lo, hi = max(j0, 0), min(t0 + THALF + W, T)
        kTs[:, lo - j0 : hi - j0] = key[b].T[:, lo:hi]
        mk = np.empty((NBLK, 128, BAND), np.float32)
        for r in range(NBLK):
            R = t0 + 128 * r
            j = n + R - W
            ok = (n >= p) & (n <= p + 2 * W) & (j >= 0) & (j < T)
            mk[r] = np.where(ok, 0.0, NEG)
        # [128, NBLK, 2*BAND]: band mask duplicated for the head pair
        mk2 = np.concatenate([mk, mk], axis=2).transpose(1, 0, 2)
        in_maps.append(
            {
                "qT": qTs,
                "kT": kTs,
                "wqT": wqT,
                "wkT": wkT,
                "bia": bia,
                "msk": np.ascontiguousarray(mk2),
            }
        )
    return in_maps


def host_gather(results):
    """results: list of 8 dicts with 'outp' [128, NBLK, NH, BAND] ->
    full output [B, NH, T, WIN]."""
    band = np.empty((B, NH, 16, 128, BAND), np.float32)
    for c in range(NCORES):
        b, th = c // 2, c % 2
        # [p, r, h, n] -> [h, r, p, n]
        band[b, :, th * NBLK : (th + 1) * NBLK] = results[c]["outp"].transpose(
            2, 1, 0, 3
        )
    band = band.reshape(B, NH, T, BAND)
    i = np.arange(T)
    nstart = np.clip(i - W, 0, T - WIN) - (128 * (i // 128) - W)
    idx = nstart[:, None] + np.arange(WIN)[None, :]  # [T, WIN]
    out = np.take_along_axis(band, idx[None, None, :, :], axis=-1)
    return np.ascontiguousarray(out)


def kernel(query, key, Wq, bq, Wk, bk):
    from concourse import bass_utils

    nc = _get_nc()
    in_maps = host_prep(query, key, Wq, bq, Wk, bk)
    res = bass_utils.run_bass_kernel_spmd(nc, in_maps, core_ids=list(range(NCORES)))
    return host_gather(res.results)



# revision 6
# speedup vs baseline: 3.0954x; 3.0954x over previous
"""Banded multi-head attention kernel for Trainium2 (8 NeuronCores).

Problem: q = query @ Wq.T + bq, k = key @ Wk.T + bk  (per head, dk=64),
scores = q.k / sqrt(dk) masked to |i-j| <= 16, softmax over keys, then
gather the 33-column select window per row -> out [B, NH, T, 33].

Strategy (v2, fp16 + 64-wide strip bands):
  - Shard (batch b, half of T) across the 8 cores; each core computes all
    8 heads for its 1024 query rows.
  - All matmuls in float16 (1 cycle/col on the PE vs 4 for fp32
    LOW_HIGH multipass; fp32 PSUM accumulation).  End-to-end rel err
    ~1.5e-3 (simulated + verified) vs the 2e-2 gate.
  - Scores in a 64-wide band: each 128-row block is split into four
    32-row strips whose key windows are 64 wide (vs 160 for the whole
    block).  8 heads x 4 strips = 32 matmuls [K=64,M=32,N=64] pack into
    one 512-col PSUM bank via 4-way column tiling (tile_position).
    All score matmuls use PE array rows 0-63 only: mixing row groups
    across column-sharing matmuls faults the device (measured), so the
    odd heads' projections (PSUM partitions 64-127) are moved to
    partitions 0-63 with SBUF->SBUF DMAs first.
  - The band mask (-60000 outside the window) initializes the PSUM bank
    via one full matmul ident.T @ mask (start=True); the 32 strip
    matmuls accumulate on top (per-element has_written semantics).
    ScalarE then does exp(x/8) PSUM->SBUF in ONE 512-wide op per block;
    DVE does one 512-wide row-sum reduce.  No per-head elementwise ops.
  - Normalization (divide by row sums) happens on the host, which also
    corrects the denominators of the <=16 global edge rows exactly
    (out-of-range keys are zero-padded -> each contributes exactly 1.0).
  - Host: final diagonal gather band -> [T, 33] + divide.
"""

import sys

sys.path.insert(0, "/opt/trn_rl_repo")

import numpy as np

B, T, HID = 4, 2048, 512
NH, DK, W = 8, 64, 16
WIN = 2 * W + 1  # 33
TEMP = 8.0
NCORES = 8
THALF = T // 2  # rows per core
NBLK = THALF // 128  # 8 row blocks per core
SB = 64  # strip band width: 32-row strip -> 32 + 2*16 keys
KW = THALF + 2 * W  # 1056 k^T columns needed per core
NEG = -60000.0  # band mask value (f16-representable; exp -> 0)

_CACHE = {}


def _build_nc():
    import concourse.bass as bass  # noqa: F401
    import concourse.tile as tile
    from concourse import bacc, mybir

    f32 = mybir.dt.float32
    f16 = mybir.dt.float16
    AF = mybir.ActivationFunctionType
    AX = mybir.AxisListType
    ALU = mybir.AluOpType

    nc = bacc.Bacc("TRN2", target_bir_lowering=False, debug=False)

    qT = nc.dram_tensor("qT", [HID, THALF], f16, kind="ExternalInput").ap()
    kT = nc.dram_tensor("kT", [HID, KW], f16, kind="ExternalInput").ap()
    wqT = nc.dram_tensor("wqT", [HID, HID], f16, kind="ExternalInput").ap()
    wkT = nc.dram_tensor("wkT", [HID, HID], f16, kind="ExternalInput").ap()
    # biases: [:, 0:4] = bq chunks, [:, 4:8] = bk chunks
    bia = nc.dram_tensor("bia", [128, 8], f32, kind="ExternalInput").ap()
    # strip band mask, replicated per head: [128, NH*SB]
    msk = nc.dram_tensor("msk", [128, NH * SB], f16, kind="ExternalInput").ap()
    idn = nc.dram_tensor("idn", [128, 128], f16, kind="ExternalInput").ap()
    # exp band: [p, r, h, n]; row sums [p, r, h]
    outp = nc.dram_tensor(
        "outp", [128, NBLK, NH, SB], f32, kind="ExternalOutput"
    ).ap()
    sums = nc.dram_tensor("sums", [128, NBLK, NH], f32, kind="ExternalOutput").ap()

    with tile.TileContext(nc) as tc:
        from contextlib import ExitStack

        with ExitStack() as ctx:
            const = ctx.enter_context(tc.tile_pool(name="const", bufs=1))
            psum_p = ctx.enter_context(
                tc.tile_pool(name="psum_p", bufs=3, space="PSUM")
            )
            psum_s = ctx.enter_context(
                tc.tile_pool(name="psum_s", bufs=2, space="PSUM")
            )

            qin = const.tile([128, 4, THALF], f16, tag="qin", name="qin")
            kin = const.tile([128, 4, KW], f16, tag="kin", name="kin")
            wq_sb = const.tile([128, 4, HID], f16, tag="wqs", name="wqs")
            wk_sb = const.tile([128, 4, HID], f16, tag="wks", name="wks")
            mk_sb = const.tile([128, NH * SB], f16, tag="mks", name="mks")
            id_sb = const.tile([128, 128], f16, tag="idn", name="idn")
            bia_sb = const.tile([128, 8], f32, tag="bia", name="bias")
            # projections: [p = out-channel within oc chunk, oc, t]
            qp = const.tile([128, 4, THALF], f16, tag="qp", name="qp")
            kp = const.tile([128, 4, KW], f16, tag="kp", name="kp")
            # odd heads' dk rows moved to partitions 0-63
            qpo = const.tile([64, 4, THALF], f16, tag="qpo", name="qpo")
            kpo = const.tile([64, 4, KW], f16, tag="kpo", name="kpo")
            # persistent exp-band region [p, r, h, n] + row sums
            ob = const.tile([128, NBLK, NH, SB], f32, tag="ob", name="ob")
            sm_sb = const.tile([128, NBLK, NH], f32, tag="sms", name="sms")

            qT_r = qT.rearrange("(c p) t -> p c t", p=128)
            kT_r = kT.rearrange("(c p) t -> p c t", p=128)
            wqT_r = wqT.rearrange("(c p) o -> p c o", p=128)
            wkT_r = wkT.rearrange("(c p) o -> p c o", p=128)
            # input DMAs, ordered by first use
            nc.sync.dma_start(out=wq_sb[:, :, :], in_=wqT_r[:, :, :])
            nc.sync.dma_start(out=qin[:, :, 0:512], in_=qT_r[:, :, 0:512])
            nc.sync.dma_start(out=bia_sb[:, :], in_=bia[:, :])
            nc.sync.dma_start(out=wk_sb[:, :, :], in_=wkT_r[:, :, :])
            nc.sync.dma_start(out=kin[:, :, 0:512], in_=kT_r[:, :, 0:512])
            nc.sync.dma_start(out=id_sb[:, :], in_=idn[:, :])
            nc.sync.dma_start(out=mk_sb[:, :], in_=msk[:, :])
            nc.sync.dma_start(out=qin[:, :, 512:THALF], in_=qT_r[:, :, 512:THALF])
            nc.sync.dma_start(out=kin[:, :, 512:KW], in_=kT_r[:, :, 512:KW])

            ncopy = [0]

            def psum_to_sbuf(dst, ps_ap, bia_ap):
                # alternate psum->sbuf(+bias) copies between ScalarE and DVE
                if ncopy[0] % 2 == 0:
                    nc.scalar.activation(
                        dst, ps_ap, AF.Identity, bias=bia_ap, scale=1.0
                    )
                else:
                    nc.vector.tensor_scalar_add(dst, ps_ap, bia_ap)
                ncopy[0] += 1

            def emit_qproj(tb):
                tsl = slice(512 * tb, 512 * (tb + 1))
                for oc in range(4):
                    osl = slice(128 * oc, 128 * (oc + 1))
                    ps = psum_p.tile([128, 512], f32, tag="psp", name="psp")
                    for ic in range(4):
                        nc.tensor.matmul(
                            ps[:, :],
                            wq_sb[:, ic, osl],
                            qin[:, ic, tsl],
                            start=(ic == 0),
                            stop=(ic == 3),
                        )
                    psum_to_sbuf(qp[:, oc, tsl], ps[:, :], bia_sb[:, oc : oc + 1])
                # odd heads' 64 dk rows -> partitions 0-63 (all 4 oc at once)
                nc.sync.dma_start(
                    out=qpo[:, :, tsl], in_=qp[64:128, :, tsl]
                )

            def emit_kproj(c0, cn):
                csl = slice(c0, c0 + cn)
                for oc in range(4):
                    osl = slice(128 * oc, 128 * (oc + 1))
                    ps = psum_p.tile([128, 512], f32, tag="psp", name="psp")
                    for ic in range(4):
                        nc.tensor.matmul(
                            ps[:, :cn],
                            wk_sb[:, ic, osl],
                            kin[:, ic, csl],
                            start=(ic == 0),
                            stop=(ic == 3),
                        )
                    psum_to_sbuf(
                        kp[:, oc, csl], ps[:, :cn], bia_sb[:, 4 + oc : 5 + oc]
                    )
                nc.sync.dma_start(out=kpo[:, :, csl], in_=kp[64:128, :, csl])

            def emit_scores(r):
                ps = psum_s.tile([128, NH * SB], f32, tag="pss", name="pss")
                # initialize the whole bank with the band mask via the PE
                # (ident.T @ mask, start=True); the 32 strip matmuls then
                # accumulate on top (per-element has_written adds onto the
                # mask).  skip_group_check: the sim's zero-region tracker
                # can't express partition-sliced accumulation, but
                # per-element HW semantics are exact (verified on HW).
                nc.tensor.matmul(
                    ps[:, :], id_sb[:, :], mk_sb[:, :], start=True, stop=False,
                    skip_group_check=True,
                )
                nmm = 0
                for oc in range(4):
                    for half in range(2):
                        h = 2 * oc + half
                        for s in range(4):
                            c = 128 * r + 32 * s
                            if half == 0:
                                lhsT = qp[0:64, oc, c : c + 32]
                                rhs = kp[0:64, oc, c : c + SB]
                            else:
                                lhsT = qpo[:, oc, c : c + 32]
                                rhs = kpo[:, oc, c : c + SB]
                            nmm += 1
                            nc.tensor.matmul(
                                ps[32 * s : 32 * s + 32, SB * h : SB * (h + 1)],
                                lhsT,
                                rhs,
                                start=False,
                                stop=(nmm == 32),
                                tile_position=(0, 32 * s),
                                skip_group_check=True,
                            )
                nc.scalar.activation(
                    ob[:, r, :, :], ps[:, :], AF.Exp, scale=1.0 / TEMP
                )
                nc.vector.tensor_reduce(
                    sm_sb[:, r, :], ob[:, r, :, :], axis=AX.X, op=ALU.add
                )
                nc.sync.dma_start(out=outp[:, r, :, :], in_=ob[:, r, :, :])

            # interleave projections and score blocks so ScalarE/DVE
            # post-processing overlaps PE matmuls throughout
            emit_qproj(0)
            emit_kproj(0, 512)
            for r in range(0, 3):
                emit_scores(r)
            emit_qproj(1)
            emit_kproj(512, 512)
            for r in range(3, 7):
                emit_scores(r)
            emit_kproj(1024, KW - 1024)
            emit_scores(7)
            nc.sync.dma_start(out=sums[:, :, :], in_=sm_sb[:, :, :])

    nc.compile()
    return nc


def _get_nc():
    if "nc" not in _CACHE:
        _CACHE["nc"] = _build_nc()
    return _CACHE["nc"]


def host_prep(query, key, Wq, bq, Wk, bk):
    """Build the 8 per-core input maps."""
    query = np.asarray(query, dtype=np.float32)
    key = np.asarray(key, dtype=np.float32)
    Wq = np.asarray(Wq, dtype=np.float32)
    Wk = np.asarray(Wk, dtype=np.float32)
    bq = np.asarray(bq, dtype=np.float32)
    bk = np.asarray(bk, dtype=np.float32)

    wqT = np.ascontiguousarray(Wq.T).astype(np.float16)
    wkT = np.ascontiguousarray(Wk.T).astype(np.float16)
    bia = np.ascontiguousarray(
        np.concatenate([bq.reshape(4, 128).T, bk.reshape(4, 128).T], axis=1)
    )
    idn = np.eye(128, dtype=np.float16)

    # strip band mask [128, NH*SB]: partition p = 32*s + p', band col b;
    # in-window iff 0 <= b - p' <= 2W  (global edges fixed on host)
    p = np.arange(128)
    pp = p % 32
    b_ = np.arange(SB)
    m0 = np.where(
        (b_[None, :] - pp[:, None] >= 0) & (b_[None, :] - pp[:, None] <= 2 * W),
        0.0,
        NEG,
    ).astype(np.float16)
    mk = np.ascontiguousarray(np.tile(m0, (1, NH)))  # [128, NH*SB]

    in_maps = []
    for c in range(NCORES):
        b, th = c // 2, c % 2
        t0 = th * THALF
        qTs = np.ascontiguousarray(query[b].T[:, t0 : t0 + THALF]).astype(
            np.float16
        )
        kTs = np.zeros((HID, KW), np.float16)
        j0 = t0 - W
        lo, hi = max(j0, 0), min(t0 + THALF + W, T)
        kTs[:, lo - j0 : hi - j0] = key[b].T[:, lo:hi].astype(np.float16)
        in_maps.append(
            {
                "qT": qTs,
                "kT": kTs,
                "wqT": wqT,
                "wkT": wkT,
                "bia": bia,
                "msk": mk,
                "idn": idn,
            }
        )
    return in_maps


def host_gather(results):
    """results: list of 8 dicts with 'outp' [128, NBLK, NH, SB] and
    'sums' [128, NBLK, NH] -> full output [B, NH, T, WIN]."""
    band = np.empty((B, NH, T, SB), np.float32)
    den = np.empty((B, NH, T), np.float32)
    for c in range(NCORES):
        b, th = c // 2, c % 2
        t0 = th * THALF
        # [p, r, h, n] -> [h, r, p, n] -> [h, r*128+p, n]
        band[b, :, t0 : t0 + THALF] = (
            results[c]["outp"].transpose(2, 1, 0, 3).reshape(NH, THALF, SB)
        )
        den[b, :, t0 : t0 + THALF] = (
            results[c]["sums"].transpose(2, 1, 0).reshape(NH, THALF)
        )
    # exact denominator correction for global edge rows: out-of-range keys
    # are zero-padded -> score 0 -> exp contributes exactly 1.0 each
    i = np.arange(T)
    n_inv = np.maximum(0, W - i) + np.maximum(0, i - (T - 1 - W))
    den -= n_inv[None, None, :].astype(np.float32)
    # gather the select window from the strip band
    g0 = np.clip(i - W, 0, T - WIN)
    c0 = g0 - i + (i % 32) + W  # start col within the 64-wide strip band
    idx = c0[:, None] + np.arange(WIN)[None, :]  # [T, WIN]
    out = np.take_along_axis(band, idx[None, None, :, :], axis=-1)
    out /= den[..., None]
    return np.ascontiguousarray(out)


def kernel(query, key, Wq, bq, Wk, bk):
    from concourse import bass_utils

    nc = _get_nc()
    in_maps = host_prep(query, key, Wq, bq, Wk, bk)
    res = bass_utils.run_bass_kernel_spmd(nc, in_maps, core_ids=list(range(NCORES)))
    return host_gather(res.results)
